# revision 5
# baseline (speedup 1.0000x reference)
"""AdaLoRA linear layer on 8 TRN2 NeuronCores.

Computes y = x @ (W + s * (P*Lambda*mask) @ Q)^T for
x[8192,4096], W[4096,4096], P[4096,64], Q[64,4096], s=2.0.

Strategy: data-parallel over the 8192 token dim (1024 tokens/core).
Each core computes its y shard with a single fused pass:
  t   = x_s @ Q^T                  (rank-64, tiny)
  y   = t @ Ptilde^T + x_s @ W^T   (one PSUM accumulation group per tile)
where Ptilde = P * (s*Lambda*mask) is folded on the host.

All device inputs are pre-cast to bf16 AND pre-tiled on the host into
the exact SBUF layout (partition-major [p, kb, free]) so every DMA is a
fat contiguous copy at full HBM bandwidth. f32 would double the DMA
bytes and starve the PE during warmup (measured: 27us head + 30us of
startup gaps + HAM clock-throttle until ~95us).
"""

import os
import sys
import time
import types

for _p in ("/opt/trn_rl_repo", "/opt/pypackages"):
    if os.path.isdir(_p) and _p not in sys.path:
        sys.path.append(_p)

# antenv.axon_hooks is not shipped in this image, but bass_utils imports it
# when trace=True under axon. If it's genuinely missing, register a shim
# backed by the real ctypes NTFF hook so tracing still works.
try:
    import antenv.axon_hooks  # noqa: F401
except Exception:
    _mod = types.ModuleType("antenv.axon_hooks")
    _mod._hook = None

    def _set_hook(h, _m=_mod):
        _m._hook = h

    def _get_hook(_m=_mod):
        return _m._hook

    _mod.set_axon_ntff_profile_hook = _set_hook
    _mod.get_axon_ntff_profile_hook = _get_hook
    try:
        from trn_agent_boot.trn_boot import _ntff_profile_via_ctypes

        _mod._hook = _ntff_profile_via_ctypes("/opt/axon/libaxon_pjrt.so")
    except Exception:
        pass
    sys.modules["antenv.axon_hooks"] = _mod

import ml_dtypes
import numpy as np

import concourse.mybir as mybir
import concourse.tile as tile
from concourse import bacc
from concourse.bass_utils import run_bass_kernel_spmd
from concourse.tile_rust import add_dep_helper

N_CORES = 8
IN_F = 4096
OUT_F = 4096
RANK = 64
BT = 8192
M_PER = BT // N_CORES  # 1024 tokens per core
SCALING = 2.0

P_DIM = 128
KB = IN_F // P_DIM  # 32 k-blocks
MS = M_PER // P_DIM  # 8 m-subtiles per core
N_STRIPE = 512
NS = OUT_F // N_STRIPE  # 8 n-stripes

XH = 512  # x chunk width (m)
NXC = M_PER // XH  # 2 chunks
MPC = XH // P_DIM  # 4 m-subtiles per x chunk

BF16 = ml_dtypes.bfloat16

_graph_cache = None


def _build_graph():
    f32 = mybir.dt.float32
    bf16 = mybir.dt.bfloat16

    nc = bacc.Bacc(None, target_bir_lowering=False, debug=False)

    # All inputs pre-tiled host-side, bf16.
    xt = nc.declare_dram_parameter("xt", [NXC, P_DIM, KB, XH], bf16, isOutput=False)
    wt = nc.declare_dram_parameter("wt", [NS, P_DIM, KB, N_STRIPE], bf16, isOutput=False)
    qtp = nc.declare_dram_parameter("qtp", [P_DIM, KB, RANK], bf16, isOutput=False)
    pt = nc.declare_dram_parameter("pt", [P_DIM, OUT_F], bf16, isOutput=False)
    out = nc.declare_dram_parameter("out", [M_PER, OUT_F], f32, isOutput=True)

    with tile.TileContext(nc) as tc:
        with (
            tc.tile_pool(name="const", bufs=1) as constp,
            tc.tile_pool(name="xpool", bufs=1) as xpool,
            tc.tile_pool(name="wpool", bufs=2) as wpool,
            tc.tile_pool(name="ypool", bufs=3) as ypool,
            tc.tile_pool(name="psum_y", bufs=6, space="PSUM") as psum_y_pool,
            tc.tile_pool(name="psum_t", bufs=2, space="PSUM") as psum_t_pool,
        ):
            # ---- constants ----
            qt_sb = constp.tile([P_DIM, KB, RANK], bf16)
            dma_qt = nc.gpsimd.dma_start(out=qt_sb[:], in_=qtp[:])

            # Ptilde^T, host-zero-padded to 128 partitions (rows 64..127)
            pt_sb = constp.tile([P_DIM, OUT_F], bf16)
            dma_pt = nc.gpsimd.dma_start(out=pt_sb[:], in_=pt[:])

            # t^T per chunk, zero-padded to 128 partitions
            tT_all = constp.tile([P_DIM, NXC, XH], bf16)
            nc.vector.memset(tT_all[:], 0.0)

            # ---- x^T resident in SBUF, 2 m-chunks; chunk0 in 4 kb-slabs
            # (fine-grained so compute starts as soon as slabs land) ----
            xts = []
            xdmas = []  # per chunk, list of slab dmas
            for h in range(NXC):
                xt_h = xpool.tile(
                    [P_DIM, KB, XH], bf16, name=f"xt_h{h}", tag=f"xt_h{h}"
                )
                nslab = 4 if h == 0 else 2
                step = KB // nslab
                ds = []
                for s_ in range(nslab):
                    ksl = slice(s_ * step, (s_ + 1) * step)
                    ds.append(
                        nc.gpsimd.dma_start(
                            out=xt_h[:, ksl, :], in_=xt[h, :, ksl, :]
                        )
                    )
                xts.append(xt_h)
                xdmas.append(ds)

            # ---- weight stripes (512 wide); stripe0 in 4 kb-slabs ----
            stripes = []
            wdmas = []
            for ns in range(NS):
                wt_sb = wpool.tile(
                    [P_DIM, KB, N_STRIPE], bf16, tag="wt_sb", name=f"wt_sb{ns}"
                )
                nslab = 4 if ns == 0 else 2
                step = KB // nslab
                ds = []
                for s_ in range(nslab):
                    ksl = slice(s_ * step, (s_ + 1) * step)
                    ds.append(
                        nc.gpsimd.dma_start(
                            out=wt_sb[:, ksl, :], in_=wt[ns, :, ksl, :]
                        )
                    )
                stripes.append(wt_sb)
                wdmas.append(ds)

            # DMA ordering: x and w slabs for the first group's kb-eighths
            # arrive paired so the PE can start after just 2.1MB; a narrow
            # window early keeps the critical path short, wider later for
            # bandwidth.
            seq = [
                xdmas[0][0], wdmas[0][0], xdmas[0][1], wdmas[0][1],
                dma_qt,
                xdmas[0][2], wdmas[0][2], xdmas[0][3], wdmas[0][3],
                dma_pt,
                xdmas[1][0], xdmas[1][1], wdmas[1][0], wdmas[1][1],
            ]
            for ns in range(2, NS):
                seq += wdmas[ns]
            for i in range(len(seq)):
                win = 3 if i < 10 else 5
                if i >= win:
                    add_dep_helper(
                        seq[i].ins, seq[i - win].ins, reason="dma window order"
                    )

            # ---- compute ----
            t_psums = [None, None]

            def t_half(h, half):
                # t^T[:, h] partial: contraction over kb half into psum.
                if half == 0:
                    t_psums[h] = psum_t_pool.tile(
                        [RANK, XH], f32, tag="psum_tT", name=f"psum_tT{h}"
                    )
                psum_tT = t_psums[h]
                for kb in range(half * (KB // 2), (half + 1) * (KB // 2)):
                    nc.tensor.matmul(
                        psum_tT[:],
                        lhsT=qt_sb[:, kb, :],
                        rhs=xts[h][:, kb, :],
                        start=(kb == 0),
                        stop=(kb == KB - 1),
                    )
                if half == 1:
                    nc.scalar.copy(out=tT_all[0:RANK, h, :], in_=psum_tT[:])

            def mg_k(ws, ms, ypsum, k0, k1):
                h = ms // MPC
                mo = ms % MPC
                msl = slice(mo * P_DIM, (mo + 1) * P_DIM)
                for kb in range(k0, k1):
                    nc.tensor.matmul(
                        ypsum[:],
                        lhsT=xts[h][:, kb, msl],
                        rhs=stripes[ws][:, kb, :],
                        start=(kb == 0),
                        stop=False,
                    )

            def mg_tail(ws, ms, ypsum):
                h = ms // MPC
                mo = ms % MPC
                msl = slice(mo * P_DIM, (mo + 1) * P_DIM)
                off = ws * N_STRIPE
                nc.tensor.matmul(
                    ypsum[:],
                    lhsT=tT_all[:, h, msl],
                    rhs=pt_sb[:, off : off + N_STRIPE],
                    start=False,
                    stop=True,
                )
                y_sb = ypool.tile([P_DIM, N_STRIPE], f32, tag="y_sb", name="y_sb")
                nc.vector.tensor_copy(out=y_sb[:], in_=ypsum[:])
                nc.sync.dma_start(
                    out=out[ms * P_DIM : (ms + 1) * P_DIM, off : off + N_STRIPE],
                    in_=y_sb[:],
                )

            def new_ypsum():
                return psum_y_pool.tile(
                    [P_DIM, N_STRIPE], f32, tag="ypsum", name="ypsum"
                )

            def mg_full(ws, ms):
                yp = new_ypsum()
                mg_k(ws, ms, yp, 0, KB)
                mg_tail(ws, ms, yp)

            # PE pre-warm: dummy matmuls on the zeroed tT_all region while
            # the first input DMAs are in flight. Keeps the PE busy from
            # ~0.5us so the HAM clock ramp finishes before real work, and
            # no >3.4us idle gap re-throttles it.
            warm_ps = psum_t_pool.tile(
                [RANK, XH], f32, tag="psum_tT", name="warm_ps"
            )
            for _ in range(24):
                nc.tensor.matmul(
                    warm_ps[:],
                    lhsT=tT_all[:, 0, 0:RANK],
                    rhs=tT_all[:, 0, :],
                    start=True,
                    stop=True,
                )

            # PE order: interleave group(0,0) kb-eighths with t-phase(0)
            # halves so the first matmul only needs x0 kb0-7 + w0 kb0-7;
            # t-phase(1) rides along with groups (0,2) and (0,3) (its x
            # chunk lands later; it is first read by the tail of (0,4)).
            yp00 = new_ypsum()
            mg_k(0, 0, yp00, 0, KB // 4)
            mg_k(0, 0, yp00, KB // 4, KB // 2)
            t_half(0, 0)
            mg_k(0, 0, yp00, KB // 2, KB)
            t_half(0, 1)
            mg_tail(0, 0, yp00)

            mg_full(0, 1)

            yp02 = new_ypsum()
            mg_k(0, 2, yp02, 0, KB // 2)
            t_half(1, 0)
            mg_k(0, 2, yp02, KB // 2, KB)
            mg_tail(0, 2, yp02)

            yp03 = new_ypsum()
            mg_k(0, 3, yp03, 0, KB // 2)
            t_half(1, 1)
            mg_k(0, 3, yp03, KB // 2, KB)
            mg_tail(0, 3, yp03)

            for ms in range(4, MS):
                mg_full(0, ms)
            for ws in range(1, NS):
                for ms in range(MS):
                    mg_full(ws, ms)

    nc.compile()
    return nc


def _get_graph():
    global _graph_cache
    if _graph_cache is None:
        _graph_cache = _build_graph()
    return _graph_cache


def _prep_inputs(inputs):
    """Host-side: fold scaling into P, cast to bf16, pre-tile to SBUF layout."""
    x = np.asarray(inputs["x"], dtype=np.float32)
    weight = np.asarray(inputs["weight"], dtype=np.float32)
    P = np.asarray(inputs["P"], dtype=np.float32)
    Lambda = np.asarray(inputs["Lambda"], dtype=np.float32)
    Q = np.asarray(inputs["Q"], dtype=np.float32)
    rank_mask = np.asarray(inputs["rank_mask"])

    # Ptilde = P * (s * Lambda * mask); pad rows 64..127 of Ptilde^T with 0
    scale = (SCALING * Lambda * rank_mask.astype(np.float32)).astype(np.float32)
    ptil = (P * scale[None, :]).T  # [RANK, OUT_F]
    pt = np.zeros((P_DIM, OUT_F), dtype=BF16)
    pt[:RANK] = ptil.astype(BF16)

    # Q^T pre-tiled partition-major: [p, kb, r]
    qtp = np.ascontiguousarray(
        Q.T.astype(BF16).reshape(KB, P_DIM, RANK).transpose(1, 0, 2)
    )

    # W pre-tiled per stripe: wt[ns, p, kb, n] = W[ns*512+n, kb*128+p]
    wt = np.ascontiguousarray(
        weight.astype(BF16)
        .T.reshape(KB, P_DIM, NS, N_STRIPE)
        .transpose(2, 1, 0, 3)
    )

    # x per core, per chunk: xt[h, p, kb, m] = x_core[h*512+m, kb*128+p]
    xb = x.astype(BF16)
    in_maps = []
    for c in range(N_CORES):
        xc = xb[c * M_PER : (c + 1) * M_PER]  # [1024, 4096]
        xtc = np.ascontiguousarray(
            xc.reshape(NXC, XH, KB, P_DIM).transpose(0, 3, 2, 1)
        )
        in_maps.append({"xt": xtc, "wt": wt, "qtp": qtp, "pt": pt})
    return in_maps


def run_full(inputs, trace=False, trace_kwargs=None):
    """Run the SPMD kernel on 8 cores. Returns (y_full, BassKernelResults)."""
    in_maps = _prep_inputs(inputs)

    nc = _get_graph()
    last_err = None
    for attempt in range(3):
        try:
            res = run_bass_kernel_spmd(
                nc,
                in_maps,
                core_ids=list(range(N_CORES)),
                trace=trace,
                **(trace_kwargs or {}),
            )
            break
        except Exception as e:  # transient NRT device faults recover on retry
            last_err = e
            time.sleep(10)
    else:
        raise last_err
    y = np.concatenate([res.results[c]["out"] for c in range(N_CORES)], axis=0)
    return y.astype(np.float32, copy=False), res


def _device_available():
    try:
        import jax

        return any("NC" in str(d) or "axon" in str(d).lower() for d in jax.devices())
    except Exception:
        return False


def _run_in_subprocess(inputs):
    # The caller's process may have initialized jax on another platform
    # (e.g. JAX_PLATFORMS=cpu for the reference); run the device pass in a
    # clean child process where jax can pick up the axon/neuron backend.
    import pickle
    import subprocess
    import tempfile

    with tempfile.TemporaryDirectory() as td:
        in_path = os.path.join(td, "in.pkl")
        out_path = os.path.join(td, "out.npy")
        with open(in_path, "wb") as f:
            pickle.dump({k: np.asarray(v) for k, v in inputs.items()}, f)
        env = dict(os.environ)
        env.pop("JAX_PLATFORMS", None)
        env["KERNEL_NO_SUBPROC"] = "1"
        code = (
            "import sys, pickle, numpy as np; "
            f"sys.path.insert(0, {os.path.dirname(os.path.abspath(__file__))!r}); "
            "import kernel; "
            f"inputs = pickle.load(open({in_path!r}, 'rb')); "
            "y, _ = kernel.run_full(inputs, trace=False); "
            f"np.save({out_path!r}, y)"
        )
        subprocess.run([sys.executable, "-c", code], env=env, check=True)
        return np.load(out_path)


def kernel(**inputs) -> np.ndarray:
    if os.environ.get("KERNEL_NO_SUBPROC") != "1":
        if not _device_available():
            return _run_in_subprocess(inputs)
        try:
            y, _ = run_full(inputs, trace=False)
            return y
        except Exception:
            # A wedged device / PJRT client recovers in a fresh process
            # (observed empirically); retry once out-of-process.
            return _run_in_subprocess(inputs)
    y, _ = run_full(inputs, trace=False)
    return y


# revision 8
# speedup vs baseline: 1.0043x; 1.0043x over previous
"""AdaLoRA linear layer on 8 TRN2 NeuronCores.

Computes y = x @ (W + s * (P*Lambda*mask) @ Q)^T for
x[8192,4096], W[4096,4096], P[4096,64], Q[64,4096], s=2.0.

Strategy: data-parallel over the 8192 token dim (1024 tokens/core).
Each core computes its y shard with a single fused pass:
  t   = x_s @ Q^T                  (rank-64, tiny)
  y   = t @ Ptilde^T + x_s @ W^T   (one PSUM accumulation group per tile)
where Ptilde = P * (s*Lambda*mask) is folded on the host.

All device inputs are pre-cast to bf16 AND pre-tiled on the host into
the exact SBUF layout (partition-major [p, kb, free]) so every DMA is a
fat contiguous copy at full HBM bandwidth. f32 would double the DMA
bytes and starve the PE during warmup (measured: 27us head + 30us of
startup gaps + HAM clock-throttle until ~95us).
"""

import os
import sys
import time
import types

for _p in ("/opt/trn_rl_repo", "/opt/pypackages"):
    if os.path.isdir(_p) and _p not in sys.path:
        sys.path.append(_p)

# antenv.axon_hooks is not shipped in this image, but bass_utils imports it
# when trace=True under axon. If it's genuinely missing, register a shim
# backed by the real ctypes NTFF hook so tracing still works.
try:
    import antenv.axon_hooks  # noqa: F401
except Exception:
    _mod = types.ModuleType("antenv.axon_hooks")
    _mod._hook = None

    def _set_hook(h, _m=_mod):
        _m._hook = h

    def _get_hook(_m=_mod):
        return _m._hook

    _mod.set_axon_ntff_profile_hook = _set_hook
    _mod.get_axon_ntff_profile_hook = _get_hook
    try:
        from trn_agent_boot.trn_boot import _ntff_profile_via_ctypes

        _mod._hook = _ntff_profile_via_ctypes("/opt/axon/libaxon_pjrt.so")
    except Exception:
        pass
    sys.modules["antenv.axon_hooks"] = _mod

import ml_dtypes
import numpy as np

import concourse.mybir as mybir
import concourse.tile as tile
from concourse import bacc
from concourse.bass_utils import run_bass_kernel_spmd
from concourse.tile_rust import add_dep_helper

N_CORES = 8
IN_F = 4096
OUT_F = 4096
RANK = 64
BT = 8192
M_PER = BT // N_CORES  # 1024 tokens per core
SCALING = 2.0

P_DIM = 128
KB = IN_F // P_DIM  # 32 k-blocks
MS = M_PER // P_DIM  # 8 m-subtiles per core
N_STRIPE = 512
NS = OUT_F // N_STRIPE  # 8 n-stripes

XH = 512  # x chunk width (m)
NXC = M_PER // XH  # 2 chunks
MPC = XH // P_DIM  # 4 m-subtiles per x chunk

BF16 = ml_dtypes.bfloat16

_graph_cache = None


def _build_graph():
    f32 = mybir.dt.float32
    bf16 = mybir.dt.bfloat16

    nc = bacc.Bacc(None, target_bir_lowering=False, debug=False)

    # All inputs pre-tiled host-side, bf16.
    xt = nc.declare_dram_parameter("xt", [NXC, P_DIM, KB, XH], bf16, isOutput=False)
    wt = nc.declare_dram_parameter("wt", [NS, P_DIM, KB, N_STRIPE], bf16, isOutput=False)
    qtp = nc.declare_dram_parameter("qtp", [P_DIM, KB, RANK], bf16, isOutput=False)
    pt = nc.declare_dram_parameter("pt", [P_DIM, OUT_F], bf16, isOutput=False)
    out = nc.declare_dram_parameter("out", [M_PER, OUT_F], f32, isOutput=True)

    with tile.TileContext(nc) as tc:
        with (
            tc.tile_pool(name="const", bufs=1) as constp,
            tc.tile_pool(name="xpool", bufs=1) as xpool,
            tc.tile_pool(name="wpool", bufs=2) as wpool,
            tc.tile_pool(name="ypool", bufs=3) as ypool,
            tc.tile_pool(name="psum_y", bufs=6, space="PSUM") as psum_y_pool,
            tc.tile_pool(name="psum_t", bufs=2, space="PSUM") as psum_t_pool,
        ):
            # ---- constants ----
            qt_sb = constp.tile([P_DIM, KB, RANK], bf16)
            dma_qt = nc.gpsimd.dma_start(out=qt_sb[:], in_=qtp[:])

            # Ptilde^T, host-zero-padded to 128 partitions (rows 64..127)
            pt_sb = constp.tile([P_DIM, OUT_F], bf16)
            dma_pt = nc.gpsimd.dma_start(out=pt_sb[:], in_=pt[:])

            # t^T per chunk, zero-padded to 128 partitions
            tT_all = constp.tile([P_DIM, NXC, XH], bf16)
            nc.vector.memset(tT_all[:], 0.0)

            # ---- x^T resident in SBUF, 2 m-chunks; chunk0 in 4 kb-slabs
            # (fine-grained so compute starts as soon as slabs land) ----
            xts = []
            xdmas = []  # per chunk, list of slab dmas
            for h in range(NXC):
                xt_h = xpool.tile(
                    [P_DIM, KB, XH], bf16, name=f"xt_h{h}", tag=f"xt_h{h}"
                )
                nslab = 4 if h == 0 else 2
                step = KB // nslab
                ds = []
                for s_ in range(nslab):
                    ksl = slice(s_ * step, (s_ + 1) * step)
                    ds.append(
                        nc.gpsimd.dma_start(
                            out=xt_h[:, ksl, :], in_=xt[h, :, ksl, :]
                        )
                    )
                xts.append(xt_h)
                xdmas.append(ds)

            # ---- weight stripes (512 wide); stripe0 in 4 kb-slabs ----
            stripes = []
            wdmas = []
            for ns in range(NS):
                wt_sb = wpool.tile(
                    [P_DIM, KB, N_STRIPE], bf16, tag="wt_sb", name=f"wt_sb{ns}"
                )
                nslab = 4 if ns == 0 else 2
                step = KB // nslab
                ds = []
                for s_ in range(nslab):
                    ksl = slice(s_ * step, (s_ + 1) * step)
                    ds.append(
                        nc.gpsimd.dma_start(
                            out=wt_sb[:, ksl, :], in_=wt[ns, :, ksl, :]
                        )
                    )
                stripes.append(wt_sb)
                wdmas.append(ds)

            # DMA ordering: x and w kb-slabs arrive as lockstep pairs (the
            # compute consumes them kb-quarter by kb-quarter); narrow
            # window early keeps the critical path short, wider later for
            # bandwidth.
            seq = [
                xdmas[0][0], wdmas[0][0], xdmas[0][1], wdmas[0][1],
                dma_qt,
                xdmas[0][2], wdmas[0][2], xdmas[0][3], wdmas[0][3],
                dma_pt,
                xdmas[1][0], xdmas[1][1], wdmas[1][0], wdmas[1][1],
            ]
            for ns in range(2, NS):
                seq += wdmas[ns]
            for i in range(len(seq)):
                win = 2 if i < 9 else 5
                if i >= win:
                    add_dep_helper(
                        seq[i].ins, seq[i - win].ins, reason="dma window order"
                    )

            # ---- compute ----
            t_psums = [None, None]

            def t_half(h, half):
                # t^T[:, h] partial: contraction over kb half into psum.
                if half == 0:
                    t_psums[h] = psum_t_pool.tile(
                        [RANK, XH], f32, tag="psum_tT", name=f"psum_tT{h}"
                    )
                psum_tT = t_psums[h]
                for kb in range(half * (KB // 2), (half + 1) * (KB // 2)):
                    nc.tensor.matmul(
                        psum_tT[:],
                        lhsT=qt_sb[:, kb, :],
                        rhs=xts[h][:, kb, :],
                        start=(kb == 0),
                        stop=(kb == KB - 1),
                    )
                if half == 1:
                    nc.scalar.copy(out=tT_all[0:RANK, h, :], in_=psum_tT[:])

            def mg_k(ws, ms, ypsum, k0, k1):
                h = ms // MPC
                mo = ms % MPC
                msl = slice(mo * P_DIM, (mo + 1) * P_DIM)
                for kb in range(k0, k1):
                    nc.tensor.matmul(
                        ypsum[:],
                        lhsT=xts[h][:, kb, msl],
                        rhs=stripes[ws][:, kb, :],
                        start=(kb == 0),
                        stop=False,
                    )

            def mg_tail(ws, ms, ypsum):
                h = ms // MPC
                mo = ms % MPC
                msl = slice(mo * P_DIM, (mo + 1) * P_DIM)
                off = ws * N_STRIPE
                nc.tensor.matmul(
                    ypsum[:],
                    lhsT=tT_all[:, h, msl],
                    rhs=pt_sb[:, off : off + N_STRIPE],
                    start=False,
                    stop=True,
                )
                y_sb = ypool.tile([P_DIM, N_STRIPE], f32, tag="y_sb", name="y_sb")
                nc.vector.tensor_copy(out=y_sb[:], in_=ypsum[:])
                nc.sync.dma_start(
                    out=out[ms * P_DIM : (ms + 1) * P_DIM, off : off + N_STRIPE],
                    in_=y_sb[:],
                )

            def new_ypsum():
                return psum_y_pool.tile(
                    [P_DIM, N_STRIPE], f32, tag="ypsum", name="ypsum"
                )

            def mg_full(ws, ms):
                yp = new_ypsum()
                mg_k(ws, ms, yp, 0, KB)
                mg_tail(ws, ms, yp)

            # PE pre-warm: dummy matmuls on the zeroed tT_all region while
            # the first input DMAs are in flight. Keeps the PE busy from
            # ~0.5us so the HAM clock ramp finishes before real work, and
            # no >3.4us idle gap re-throttles it.
            warm_ps = psum_t_pool.tile(
                [RANK, XH], f32, tag="psum_tT", name="warm_ps"
            )
            for _ in range(16):
                nc.tensor.matmul(
                    warm_ps[:],
                    lhsT=tT_all[:, 0, 0:RANK],
                    rhs=tT_all[:, 0, :],
                    start=True,
                    stop=True,
                )

            # PE order for the startup phase: interleave the four chunk-0
            # m-groups kb-quarter-wise so PE consumption of each arriving
            # (x,w) slab pair (~2.1MB per 6.9us of compute) matches the
            # DMA delivery rate; one group alone would drain a slab 3x
            # faster than HBM supplies it and stall. t-phase(0) slots in
            # once qt has landed; t-phase(1) rides with groups (0,5/6).
            KQ = KB // 4
            yps = [new_ypsum() for _ in range(MPC)]
            for q in range(4):
                for ms in range(MPC):
                    mg_k(0, ms, yps[ms], q * KQ, (q + 1) * KQ)
                if q == 2:
                    t_half(0, 0)
            t_half(0, 1)
            for ms in range(MPC):
                mg_tail(0, ms, yps[ms])

            yp04 = new_ypsum()
            mg_k(0, 4, yp04, 0, KB // 2)
            t_half(1, 0)
            mg_k(0, 4, yp04, KB // 2, KB)
            t_half(1, 1)
            mg_tail(0, 4, yp04)

            for ms in range(5, MS):
                mg_full(0, ms)
            for ws in range(1, NS):
                for ms in range(MS):
                    mg_full(ws, ms)

    nc.compile()
    return nc


def _get_graph():
    global _graph_cache
    if _graph_cache is None:
        _graph_cache = _build_graph()
    return _graph_cache


def _prep_inputs(inputs):
    """Host-side: fold scaling into P, cast to bf16, pre-tile to SBUF layout."""
    x = np.asarray(inputs["x"], dtype=np.float32)
    weight = np.asarray(inputs["weight"], dtype=np.float32)
    P = np.asarray(inputs["P"], dtype=np.float32)
    Lambda = np.asarray(inputs["Lambda"], dtype=np.float32)
    Q = np.asarray(inputs["Q"], dtype=np.float32)
    rank_mask = np.asarray(inputs["rank_mask"])

    # Ptilde = P * (s * Lambda * mask); pad rows 64..127 of Ptilde^T with 0
    scale = (SCALING * Lambda * rank_mask.astype(np.float32)).astype(np.float32)
    ptil = (P * scale[None, :]).T  # [RANK, OUT_F]
    pt = np.zeros((P_DIM, OUT_F), dtype=BF16)
    pt[:RANK] = ptil.astype(BF16)

    # Q^T pre-tiled partition-major: [p, kb, r]
    qtp = np.ascontiguousarray(
        Q.T.astype(BF16).reshape(KB, P_DIM, RANK).transpose(1, 0, 2)
    )

    # W pre-tiled per stripe: wt[ns, p, kb, n] = W[ns*512+n, kb*128+p]
    wt = np.ascontiguousarray(
        weight.astype(BF16)
        .T.reshape(KB, P_DIM, NS, N_STRIPE)
        .transpose(2, 1, 0, 3)
    )

    # x per core, per chunk: xt[h, p, kb, m] = x_core[h*512+m, kb*128+p]
    xb = x.astype(BF16)
    in_maps = []
    for c in range(N_CORES):
        xc = xb[c * M_PER : (c + 1) * M_PER]  # [1024, 4096]
        xtc = np.ascontiguousarray(
            xc.reshape(NXC, XH, KB, P_DIM).transpose(0, 3, 2, 1)
        )
        in_maps.append({"xt": xtc, "wt": wt, "qtp": qtp, "pt": pt})
    return in_maps


def run_full(inputs, trace=False, trace_kwargs=None):
    """Run the SPMD kernel on 8 cores. Returns (y_full, BassKernelResults)."""
    in_maps = _prep_inputs(inputs)

    nc = _get_graph()
    last_err = None
    for attempt in range(3):
        try:
            res = run_bass_kernel_spmd(
                nc,
                in_maps,
                core_ids=list(range(N_CORES)),
                trace=trace,
                **(trace_kwargs or {}),
            )
            break
        except Exception as e:  # transient NRT device faults recover on retry
            last_err = e
            time.sleep(10)
    else:
        raise last_err
    y = np.concatenate([res.results[c]["out"] for c in range(N_CORES)], axis=0)
    return y.astype(np.float32, copy=False), res


def _device_available():
    try:
        import jax

        return any("NC" in str(d) or "axon" in str(d).lower() for d in jax.devices())
    except Exception:
        return False


def _run_in_subprocess(inputs):
    # The caller's process may have initialized jax on another platform
    # (e.g. JAX_PLATFORMS=cpu for the reference); run the device pass in a
    # clean child process where jax can pick up the axon/neuron backend.
    import pickle
    import subprocess
    import tempfile

    with tempfile.TemporaryDirectory() as td:
        in_path = os.path.join(td, "in.pkl")
        out_path = os.path.join(td, "out.npy")
        with open(in_path, "wb") as f:
            pickle.dump({k: np.asarray(v) for k, v in inputs.items()}, f)
        env = dict(os.environ)
        env.pop("JAX_PLATFORMS", None)
        env["KERNEL_NO_SUBPROC"] = "1"
        code = (
            "import sys, pickle, numpy as np; "
            f"sys.path.insert(0, {os.path.dirname(os.path.abspath(__file__))!r}); "
            "import kernel; "
            f"inputs = pickle.load(open({in_path!r}, 'rb')); "
            "y, _ = kernel.run_full(inputs, trace=False); "
            f"np.save({out_path!r}, y)"
        )
        subprocess.run([sys.executable, "-c", code], env=env, check=True)
        return np.load(out_path)


def kernel(**inputs) -> np.ndarray:
    if os.environ.get("KERNEL_NO_SUBPROC") != "1":
        if not _device_available():
            return _run_in_subprocess(inputs)
        try:
            y, _ = run_full(inputs, trace=False)
            return y
        except Exception:
            # A wedged device / PJRT client recovers in a fresh process
            # (observed empirically); retry once out-of-process.
            return _run_in_subprocess(inputs)
    y, _ = run_full(inputs, trace=False)
    return y


# revision 9
# speedup vs baseline: 1.0226x; 1.0182x over previous
"""AdaLoRA linear layer on 8 TRN2 NeuronCores.

Computes y = x @ (W + s * (P*Lambda*mask) @ Q)^T for
x[8192,4096], W[4096,4096], P[4096,64], Q[64,4096], s=2.0.

Strategy: data-parallel over the 8192 token dim (1024 tokens/core).
Each core computes its y shard with a single fused pass:
  t   = x_s @ Q^T                  (rank-64, tiny)
  y   = t @ Ptilde^T + x_s @ W^T   (one PSUM accumulation group per tile)
where Ptilde = P * (s*Lambda*mask) is folded on the host.

All device inputs are pre-cast to bf16 AND pre-tiled on the host into
the exact SBUF layout (partition-major [p, kb, free]) so every DMA is a
fat contiguous copy at full HBM bandwidth. f32 would double the DMA
bytes and starve the PE during warmup (measured: 27us head + 30us of
startup gaps + HAM clock-throttle until ~95us).
"""

import os
import sys
import time
import types

for _p in ("/opt/trn_rl_repo", "/opt/pypackages"):
    if os.path.isdir(_p) and _p not in sys.path:
        sys.path.append(_p)

# antenv.axon_hooks is not shipped in this image, but bass_utils imports it
# when trace=True under axon. If it's genuinely missing, register a shim
# backed by the real ctypes NTFF hook so tracing still works.
try:
    import antenv.axon_hooks  # noqa: F401
except Exception:
    _mod = types.ModuleType("antenv.axon_hooks")
    _mod._hook = None

    def _set_hook(h, _m=_mod):
        _m._hook = h

    def _get_hook(_m=_mod):
        return _m._hook

    _mod.set_axon_ntff_profile_hook = _set_hook
    _mod.get_axon_ntff_profile_hook = _get_hook
    try:
        from trn_agent_boot.trn_boot import _ntff_profile_via_ctypes

        _mod._hook = _ntff_profile_via_ctypes("/opt/axon/libaxon_pjrt.so")
    except Exception:
        pass
    sys.modules["antenv.axon_hooks"] = _mod

import ml_dtypes
import numpy as np

import concourse.mybir as mybir
import concourse.tile as tile
from concourse import bacc
from concourse.bass_utils import run_bass_kernel_spmd
from concourse.tile_rust import add_dep_helper

N_CORES = 8
IN_F = 4096
OUT_F = 4096
RANK = 64
BT = 8192
M_PER = BT // N_CORES  # 1024 tokens per core
SCALING = 2.0

P_DIM = 128
KB = IN_F // P_DIM  # 32 k-blocks
MS = M_PER // P_DIM  # 8 m-subtiles per core
N_STRIPE = 512
NS = OUT_F // N_STRIPE  # 8 n-stripes

XH = 512  # x chunk width (m)
NXC = M_PER // XH  # 2 chunks
MPC = XH // P_DIM  # 4 m-subtiles per x chunk

BF16 = ml_dtypes.bfloat16

_graph_cache = None


def _build_graph():
    f32 = mybir.dt.float32
    bf16 = mybir.dt.bfloat16

    nc = bacc.Bacc(None, target_bir_lowering=False, debug=False)

    # All inputs pre-tiled host-side, bf16.
    xt = nc.declare_dram_parameter("xt", [NXC, P_DIM, KB, XH], bf16, isOutput=False)
    wt = nc.declare_dram_parameter("wt", [NS, P_DIM, KB, N_STRIPE], bf16, isOutput=False)
    qtp = nc.declare_dram_parameter("qtp", [P_DIM, KB, RANK], bf16, isOutput=False)
    pt = nc.declare_dram_parameter("pt", [P_DIM, OUT_F], bf16, isOutput=False)
    out = nc.declare_dram_parameter("out", [M_PER, OUT_F], f32, isOutput=True)

    with tile.TileContext(nc) as tc:
        with (
            tc.tile_pool(name="const", bufs=1) as constp,
            tc.tile_pool(name="xpool", bufs=1) as xpool,
            tc.tile_pool(name="wpool", bufs=2) as wpool,
            tc.tile_pool(name="ypool", bufs=3) as ypool,
            tc.tile_pool(name="psum_y", bufs=6, space="PSUM") as psum_y_pool,
            tc.tile_pool(name="psum_t", bufs=2, space="PSUM") as psum_t_pool,
        ):
            # ---- tiles ----
            qt_sb = constp.tile([P_DIM, KB, RANK], bf16)
            pt_sb = constp.tile([P_DIM, OUT_F], bf16)

            # t^T per chunk, zero-padded to 128 partitions
            tT_all = constp.tile([P_DIM, NXC, XH], bf16)
            nc.vector.memset(tT_all[:], 0.0)

            xts = []
            for h in range(NXC):
                xt_h = xpool.tile(
                    [P_DIM, KB, XH], bf16, name=f"xt_h{h}", tag=f"xt_h{h}"
                )
                xts.append(xt_h)
            stripes = []
            for ns in range(NS):
                wt_sb = wpool.tile(
                    [P_DIM, KB, N_STRIPE], bf16, tag="wt_sb", name=f"wt_sb{ns}"
                )
                stripes.append(wt_sb)

            # ---- input DMAs: all on gpsimd, emitted in queue order with
            # explicit pacing deps. Startup is DMA-bound: the PE consumes
            # an (x,w) kb-quarter slab pair (~2.1MB) per ~7us of compute,
            # so pairs are chained to start when the previous x slab
            # completes — lockstep delivery, no bandwidth-stealing races.
            def xslab(h, k0, k1):
                return nc.gpsimd.dma_start(
                    out=xts[h][:, k0:k1, :], in_=xt[h, :, k0:k1, :]
                )

            def wslab(ns, k0, k1):
                return nc.gpsimd.dma_start(
                    out=stripes[ns][:, k0:k1, :], in_=wt[ns, :, k0:k1, :]
                )

            KQ = KB // 4
            dma_qt = nc.gpsimd.dma_start(out=qt_sb[:], in_=qtp[:])
            x0 = [None] * 4
            w0 = [None] * 4
            for q in range(4):
                x0[q] = xslab(0, q * KQ, (q + 1) * KQ)
                w0[q] = wslab(0, q * KQ, (q + 1) * KQ)
                if q > 0:
                    add_dep_helper(x0[q].ins, x0[q - 1].ins, reason="pace x0")
            x1 = [xslab(1, 0, KB // 2), None]
            add_dep_helper(x1[0].ins, x0[2].ins, reason="pace x1a")
            w1 = [wslab(1, 0, KB // 2), None]
            add_dep_helper(w1[0].ins, w0[3].ins, reason="pace w1a")
            x1[1] = xslab(1, KB // 2, KB)
            add_dep_helper(x1[1].ins, x0[3].ins, reason="pace x1b")
            w1[1] = wslab(1, KB // 2, KB)
            add_dep_helper(w1[1].ins, w1[0].ins, reason="pace w1b")
            dma_pt = nc.gpsimd.dma_start(out=pt_sb[:], in_=pt[:])
            add_dep_helper(dma_pt.ins, x1[0].ins, reason="pace pt")
            wtail = [w1[0], w1[1]]
            for ns in range(2, NS):
                for s_ in range(2):
                    d = wslab(ns, s_ * (KB // 2), (s_ + 1) * (KB // 2))
                    add_dep_helper(
                        d.ins, wtail[-2].ins, reason="dma window order"
                    )
                    wtail.append(d)

            # ---- compute ----
            t_psums = [None, None]

            def t_half(h, half):
                # t^T[:, h] partial: contraction over kb half into psum.
                if half == 0:
                    t_psums[h] = psum_t_pool.tile(
                        [RANK, XH], f32, tag="psum_tT", name=f"psum_tT{h}"
                    )
                psum_tT = t_psums[h]
                for kb in range(half * (KB // 2), (half + 1) * (KB // 2)):
                    nc.tensor.matmul(
                        psum_tT[:],
                        lhsT=qt_sb[:, kb, :],
                        rhs=xts[h][:, kb, :],
                        start=(kb == 0),
                        stop=(kb == KB - 1),
                    )
                if half == 1:
                    nc.scalar.copy(out=tT_all[0:RANK, h, :], in_=psum_tT[:])

            def mg_k(ws, ms, ypsum, k0, k1):
                h = ms // MPC
                mo = ms % MPC
                msl = slice(mo * P_DIM, (mo + 1) * P_DIM)
                for kb in range(k0, k1):
                    nc.tensor.matmul(
                        ypsum[:],
                        lhsT=xts[h][:, kb, msl],
                        rhs=stripes[ws][:, kb, :],
                        start=(kb == 0),
                        stop=False,
                    )

            def mg_tail(ws, ms, ypsum):
                h = ms // MPC
                mo = ms % MPC
                msl = slice(mo * P_DIM, (mo + 1) * P_DIM)
                off = ws * N_STRIPE
                nc.tensor.matmul(
                    ypsum[:],
                    lhsT=tT_all[:, h, msl],
                    rhs=pt_sb[:, off : off + N_STRIPE],
                    start=False,
                    stop=True,
                )
                y_sb = ypool.tile([P_DIM, N_STRIPE], f32, tag="y_sb", name="y_sb")
                nc.vector.tensor_copy(out=y_sb[:], in_=ypsum[:])
                nc.sync.dma_start(
                    out=out[ms * P_DIM : (ms + 1) * P_DIM, off : off + N_STRIPE],
                    in_=y_sb[:],
                )

            def new_ypsum():
                return psum_y_pool.tile(
                    [P_DIM, N_STRIPE], f32, tag="ypsum", name="ypsum"
                )

            def mg_full(ws, ms):
                yp = new_ypsum()
                mg_k(ws, ms, yp, 0, KB)
                mg_tail(ws, ms, yp)

            # PE pre-warm: dummy matmuls on the zeroed tT_all region while
            # the first input DMAs are in flight. Keeps the PE busy from
            # ~0.5us so the HAM clock ramp finishes before real work, and
            # no >3.4us idle gap re-throttles it.
            warm_ps = psum_t_pool.tile(
                [RANK, XH], f32, tag="psum_tT", name="warm_ps"
            )
            for _ in range(16):
                nc.tensor.matmul(
                    warm_ps[:],
                    lhsT=tT_all[:, 0, 0:RANK],
                    rhs=tT_all[:, 0, :],
                    start=True,
                    stop=True,
                )

            # PE order for the startup phase: interleave the four chunk-0
            # m-groups kb-quarter-wise so PE consumption of each arriving
            # (x,w) slab pair (~2.1MB per 6.9us of compute) matches the
            # DMA delivery rate; one group alone would drain a slab 3x
            # faster than HBM supplies it and stall. t-phase(0) slots in
            # once qt has landed; t-phase(1) rides with groups (0,5/6).
            KQ = KB // 4
            yps = [new_ypsum() for _ in range(MPC)]
            for q in range(4):
                for ms in range(MPC):
                    mg_k(0, ms, yps[ms], q * KQ, (q + 1) * KQ)
                if q == 2:
                    t_half(0, 0)
            t_half(0, 1)
            for ms in range(MPC):
                mg_tail(0, ms, yps[ms])

            yp04 = new_ypsum()
            mg_k(0, 4, yp04, 0, KB // 2)
            t_half(1, 0)
            mg_k(0, 4, yp04, KB // 2, KB)
            t_half(1, 1)
            mg_tail(0, 4, yp04)

            for ms in range(5, MS):
                mg_full(0, ms)
            for ws in range(1, NS):
                for ms in range(MS):
                    mg_full(ws, ms)

    nc.compile()
    return nc


def _get_graph():
    global _graph_cache
    if _graph_cache is None:
        _graph_cache = _build_graph()
    return _graph_cache


def _prep_inputs(inputs):
    """Host-side: fold scaling into P, cast to bf16, pre-tile to SBUF layout."""
    x = np.asarray(inputs["x"], dtype=np.float32)
    weight = np.asarray(inputs["weight"], dtype=np.float32)
    P = np.asarray(inputs["P"], dtype=np.float32)
    Lambda = np.asarray(inputs["Lambda"], dtype=np.float32)
    Q = np.asarray(inputs["Q"], dtype=np.float32)
    rank_mask = np.asarray(inputs["rank_mask"])

    # Ptilde = P * (s * Lambda * mask); pad rows 64..127 of Ptilde^T with 0
    scale = (SCALING * Lambda * rank_mask.astype(np.float32)).astype(np.float32)
    ptil = (P * scale[None, :]).T  # [RANK, OUT_F]
    pt = np.zeros((P_DIM, OUT_F), dtype=BF16)
    pt[:RANK] = ptil.astype(BF16)

    # Q^T pre-tiled partition-major: [p, kb, r]
    qtp = np.ascontiguousarray(
        Q.T.astype(BF16).reshape(KB, P_DIM, RANK).transpose(1, 0, 2)
    )

    # W pre-tiled per stripe: wt[ns, p, kb, n] = W[ns*512+n, kb*128+p]
    wt = np.ascontiguousarray(
        weight.astype(BF16)
        .T.reshape(KB, P_DIM, NS, N_STRIPE)
        .transpose(2, 1, 0, 3)
    )

    # x per core, per chunk: xt[h, p, kb, m] = x_core[h*512+m, kb*128+p]
    xb = x.astype(BF16)
    in_maps = []
    for c in range(N_CORES):
        xc = xb[c * M_PER : (c + 1) * M_PER]  # [1024, 4096]
        xtc = np.ascontiguousarray(
            xc.reshape(NXC, XH, KB, P_DIM).transpose(0, 3, 2, 1)
        )
        in_maps.append({"xt": xtc, "wt": wt, "qtp": qtp, "pt": pt})
    return in_maps


def run_full(inputs, trace=False, trace_kwargs=None):
    """Run the SPMD kernel on 8 cores. Returns (y_full, BassKernelResults)."""
    in_maps = _prep_inputs(inputs)

    nc = _get_graph()
    last_err = None
    for attempt in range(3):
        try:
            res = run_bass_kernel_spmd(
                nc,
                in_maps,
                core_ids=list(range(N_CORES)),
                trace=trace,
                **(trace_kwargs or {}),
            )
            break
        except Exception as e:  # transient NRT device faults recover on retry
            last_err = e
            time.sleep(10)
    else:
        raise last_err
    y = np.concatenate([res.results[c]["out"] for c in range(N_CORES)], axis=0)
    return y.astype(np.float32, copy=False), res


def _device_available():
    try:
        import jax

        return any("NC" in str(d) or "axon" in str(d).lower() for d in jax.devices())
    except Exception:
        return False


def _run_in_subprocess(inputs):
    # The caller's process may have initialized jax on another platform
    # (e.g. JAX_PLATFORMS=cpu for the reference); run the device pass in a
    # clean child process where jax can pick up the axon/neuron backend.
    import pickle
    import subprocess
    import tempfile

    with tempfile.TemporaryDirectory() as td:
        in_path = os.path.join(td, "in.pkl")
        out_path = os.path.join(td, "out.npy")
        with open(in_path, "wb") as f:
            pickle.dump({k: np.asarray(v) for k, v in inputs.items()}, f)
        env = dict(os.environ)
        env.pop("JAX_PLATFORMS", None)
        env["KERNEL_NO_SUBPROC"] = "1"
        code = (
            "import sys, pickle, numpy as np; "
            f"sys.path.insert(0, {os.path.dirname(os.path.abspath(__file__))!r}); "
            "import kernel; "
            f"inputs = pickle.load(open({in_path!r}, 'rb')); "
            "y, _ = kernel.run_full(inputs, trace=False); "
            f"np.save({out_path!r}, y)"
        )
        subprocess.run([sys.executable, "-c", code], env=env, check=True)
        return np.load(out_path)


def kernel(**inputs) -> np.ndarray:
    if os.environ.get("KERNEL_NO_SUBPROC") != "1":
        if not _device_available():
            return _run_in_subprocess(inputs)
        try:
            y, _ = run_full(inputs, trace=False)
            return y
        except Exception:
            # A wedged device / PJRT client recovers in a fresh process
            # (observed empirically); retry once out-of-process.
            return _run_in_subprocess(inputs)
    y, _ = run_full(inputs, trace=False)
    return y


# revision 10
# speedup vs baseline: 1.1513x; 1.1258x over previous
"""AdaLoRA linear layer on 8 TRN2 NeuronCores — mixed fp8/bf16 PE path.

Computes y = x @ (W + s * (P*Lambda*mask) @ Q)^T for
x[8192,4096], W[4096,4096], P[4096,64], Q[64,4096], s=2.0.

Data-parallel over tokens (1024/core). The contraction dim is split:
the first KF=8 k-blocks (1024 of 4096) run as fp8e4 DoubleRow matmuls
(2 k-blocks per instruction, 2x PE throughput — measured 216ns per
K=256 x 512 instr, same as one bf16 K=128 instr), the remaining 24
k-blocks run in bf16. Measured end-to-end rel err 1.59e-2 on the
reference inputs (gate: 2e-2); fp8 quantization error scales with
sqrt(KF/KB) so KF=8 keeps a >20% margin.

Scale folding so one PSUM accumulation group stays consistent:
  W is pre-scaled x32 on both the fp8 and bf16 sides (fp8 needs it to
  stay in e4m3 normal range; bf16 absorbs it exactly), Q x64, and
  Ptilde = P*(s*Lambda*mask) enters as Ptilde*32/64; the final
  psum->SBUF copy multiplies by 1/32 on the Activation engine.
"""

import os
import sys
import time
import types

for _p in ("/opt/trn_rl_repo", "/opt/pypackages"):
    if os.path.isdir(_p) and _p not in sys.path:
        sys.path.append(_p)

try:
    import antenv.axon_hooks  # noqa: F401
except Exception:
    _mod = types.ModuleType("antenv.axon_hooks")
    _mod._hook = None

    def _set_hook(h, _m=_mod):
        _m._hook = h

    def _get_hook(_m=_mod):
        return _m._hook

    _mod.set_axon_ntff_profile_hook = _set_hook
    _mod.get_axon_ntff_profile_hook = _get_hook
    try:
        from trn_agent_boot.trn_boot import _ntff_profile_via_ctypes

        _mod._hook = _ntff_profile_via_ctypes("/opt/axon/libaxon_pjrt.so")
    except Exception:
        pass
    sys.modules["antenv.axon_hooks"] = _mod

import ml_dtypes
import numpy as np

import concourse.mybir as mybir
import concourse.tile as tile
from concourse import bacc
from concourse.bass_utils import run_bass_kernel_spmd
from concourse.tile_rust import add_dep_helper

N_CORES = 8
IN_F = 4096
OUT_F = 4096
RANK = 64
BT = 8192
M_PER = BT // N_CORES
SCALING = 2.0

P_DIM = 128
KB = IN_F // P_DIM  # 32 k-blocks
KF = 8  # k-blocks in fp8 DoubleRow (must be even)
KFP = KF // 2  # DR instructions per group
KR = KB - KF  # bf16 k-blocks
MS = M_PER // P_DIM
N_STRIPE = 512
NS = OUT_F // N_STRIPE

XH = 512
NXC = M_PER // XH
MPC = XH // P_DIM

WSCALE = 32.0
QSCALE = 64.0

BF16 = ml_dtypes.bfloat16
F8 = ml_dtypes.float8_e4m3

_graph_cache = None


def _build_graph():
    f32 = mybir.dt.float32
    bf16 = mybir.dt.bfloat16
    f8 = mybir.dt.float8e4
    DR = mybir.MatmulPerfMode.DoubleRow

    nc = bacc.Bacc(None, target_bir_lowering=False, debug=False)

    x8d = nc.declare_dram_parameter("x8", [NXC, P_DIM, KFP, 2, XH], f8, isOutput=False)
    x16d = nc.declare_dram_parameter("x16", [NXC, P_DIM, KR, XH], bf16, isOutput=False)
    w8d = nc.declare_dram_parameter(
        "w8", [NS, P_DIM, KFP, 2, N_STRIPE], f8, isOutput=False
    )
    w16d = nc.declare_dram_parameter(
        "w16", [NS, P_DIM, KR, N_STRIPE], bf16, isOutput=False
    )
    qt8d = nc.declare_dram_parameter("qt8", [P_DIM, KFP, 2, RANK], f8, isOutput=False)
    qt16d = nc.declare_dram_parameter("qt16", [P_DIM, KR, RANK], bf16, isOutput=False)
    ptd = nc.declare_dram_parameter("pt", [P_DIM, OUT_F], bf16, isOutput=False)
    out = nc.declare_dram_parameter("out", [M_PER, OUT_F], f32, isOutput=True)

    with tile.TileContext(nc) as tc:
        with (
            tc.tile_pool(name="const", bufs=1) as constp,
            tc.tile_pool(name="xpool", bufs=1) as xpool,
            tc.tile_pool(name="wpool", bufs=2) as wpool,
            tc.tile_pool(name="ypool", bufs=3) as ypool,
            tc.tile_pool(name="psum_y", bufs=6, space="PSUM") as psum_y_pool,
            tc.tile_pool(name="psum_t", bufs=2, space="PSUM") as psum_t_pool,
        ):
            # ---- tiles ----
            qt8_sb = constp.tile([P_DIM, KFP, 2, RANK], f8)
            qt16_sb = constp.tile([P_DIM, KR, RANK], bf16)
            pt_sb = constp.tile([P_DIM, OUT_F], bf16)
            tT_all = constp.tile([P_DIM, NXC, XH], bf16)
            nc.vector.memset(tT_all[:], 0.0)

            x8s, x16s = [], []
            for h in range(NXC):
                x8_h = xpool.tile(
                    [P_DIM, KFP, 2, XH], f8, name=f"x8_h{h}", tag=f"x8_h{h}"
                )
                x16_h = xpool.tile(
                    [P_DIM, KR, XH], bf16, name=f"x16_h{h}", tag=f"x16_h{h}"
                )
                x8s.append(x8_h)
                x16s.append(x16_h)
            w8s, w16s = [], []
            for ns in range(NS):
                w8_sb = wpool.tile(
                    [P_DIM, KFP, 2, N_STRIPE], f8, tag="w8_sb", name=f"w8_sb{ns}"
                )
                w16_sb = wpool.tile(
                    [P_DIM, KR, N_STRIPE], bf16, tag="w16_sb", name=f"w16_sb{ns}"
                )
                w8s.append(w8_sb)
                w16s.append(w16_sb)

            # ---- input DMAs on gpsimd, paced pairwise with the PE's
            # kb-quarter consumption during startup ----
            KQ = KR // 3  # 8 bf16 kb per slab

            def x8slab(h):
                return nc.gpsimd.dma_start(out=x8s[h][:], in_=x8d[h])

            def x16slab(h, s_):
                sl = slice(s_ * KQ, (s_ + 1) * KQ)
                return nc.gpsimd.dma_start(
                    out=x16s[h][:, sl, :], in_=x16d[h, :, sl, :]
                )

            def w8slab(ns):
                return nc.gpsimd.dma_start(out=w8s[ns][:], in_=w8d[ns])

            def w16slab(ns, s_):
                sl = slice(s_ * KQ, (s_ + 1) * KQ)
                return nc.gpsimd.dma_start(
                    out=w16s[ns][:, sl, :], in_=w16d[ns, :, sl, :]
                )

            dma_qt8 = nc.gpsimd.dma_start(out=qt8_sb[:], in_=qt8d[:])
            dma_qt16 = nc.gpsimd.dma_start(out=qt16_sb[:], in_=qt16d[:])
            x0 = [x8slab(0), None, None, None]
            w0 = [w8slab(0), None, None, None]
            for s_ in range(3):
                x0[s_ + 1] = x16slab(0, s_)
                w0[s_ + 1] = w16slab(0, s_)
                add_dep_helper(x0[s_ + 1].ins, x0[s_].ins, reason="pace x0")
            x1 = [x8slab(1), x16slab(1, 0)]
            add_dep_helper(x1[0].ins, x0[2].ins, reason="pace x1")
            w1 = [w8slab(1), w16slab(1, 0)]
            add_dep_helper(w1[0].ins, w0[3].ins, reason="pace w1")
            x1 += [x16slab(1, 1), x16slab(1, 2)]
            add_dep_helper(x1[2].ins, x0[3].ins, reason="pace x1c")
            dma_pt = nc.gpsimd.dma_start(out=pt_sb[:], in_=ptd[:])
            add_dep_helper(dma_pt.ins, x1[0].ins, reason="pace pt")
            w1 += [w16slab(1, 1), w16slab(1, 2)]
            add_dep_helper(w1[2].ins, w1[0].ins, reason="pace w1c")
            wtail = list(w1)
            for ns in range(2, NS):
                for d in (
                    w8slab(ns),
                    w16slab(ns, 0),
                    w16slab(ns, 1),
                    w16slab(ns, 2),
                ):
                    add_dep_helper(
                        d.ins, wtail[-3].ins, reason="dma window order"
                    )
                    wtail.append(d)

            # ---- compute ----
            t_psums = [None, None]

            def t_part(h, part):
                # part 0: fp8 DR k-blocks; 1/2/3: bf16 kb slabs; 3 closes.
                if part == 0:
                    t_psums[h] = psum_t_pool.tile(
                        [RANK, XH], f32, tag="psum_tT", name=f"psum_tT{h}"
                    )
                    for j in range(KFP):
                        nc.tensor.matmul(
                            t_psums[h][:],
                            lhsT=qt8_sb[:, j, :, :],
                            rhs=x8s[h][:, j, :, :],
                            start=(j == 0),
                            stop=False,
                            perf_mode=DR,
                        )
                else:
                    for kb in range((part - 1) * KQ, part * KQ):
                        nc.tensor.matmul(
                            t_psums[h][:],
                            lhsT=qt16_sb[:, kb, :],
                            rhs=x16s[h][:, kb, :],
                            start=False,
                            stop=(kb == KR - 1),
                        )
                    if part == 3:
                        nc.scalar.copy(out=tT_all[0:RANK, h, :], in_=t_psums[h][:])

            def mg_8(ws, ms, ypsum):
                h, mo = divmod(ms, MPC)
                msl = slice(mo * P_DIM, (mo + 1) * P_DIM)
                for j in range(KFP):
                    nc.tensor.matmul(
                        ypsum[:],
                        lhsT=x8s[h][:, j, :, msl],
                        rhs=w8s[ws][:, j, :, :],
                        start=(j == 0),
                        stop=False,
                        perf_mode=DR,
                    )

            def mg_16(ws, ms, ypsum, k0, k1):
                h, mo = divmod(ms, MPC)
                msl = slice(mo * P_DIM, (mo + 1) * P_DIM)
                for kb in range(k0, k1):
                    nc.tensor.matmul(
                        ypsum[:],
                        lhsT=x16s[h][:, kb, msl],
                        rhs=w16s[ws][:, kb, :],
                        start=False,
                        stop=False,
                    )

            def mg_tail(ws, ms, ypsum):
                h, mo = divmod(ms, MPC)
                msl = slice(mo * P_DIM, (mo + 1) * P_DIM)
                off = ws * N_STRIPE
                nc.tensor.matmul(
                    ypsum[:],
                    lhsT=tT_all[:, h, msl],
                    rhs=pt_sb[:, off : off + N_STRIPE],
                    start=False,
                    stop=True,
                )
                y_sb = ypool.tile([P_DIM, N_STRIPE], f32, tag="y_sb", name="y_sb")
                nc.scalar.mul(y_sb[:], ypsum[:], 1.0 / WSCALE)
                nc.sync.dma_start(
                    out=out[ms * P_DIM : (ms + 1) * P_DIM, off : off + N_STRIPE],
                    in_=y_sb[:],
                )

            def new_ypsum():
                return psum_y_pool.tile(
                    [P_DIM, N_STRIPE], f32, tag="ypsum", name="ypsum"
                )

            def mg_full(ws, ms):
                yp = new_ypsum()
                mg_8(ws, ms, yp)
                mg_16(ws, ms, yp, 0, KR)
                mg_tail(ws, ms, yp)

            # PE pre-warm on the zeroed tT region while first DMAs fly.
            warm_ps = psum_t_pool.tile(
                [RANK, XH], f32, tag="psum_tT", name="warm_ps"
            )
            for _ in range(12):
                nc.tensor.matmul(
                    warm_ps[:],
                    lhsT=tT_all[:, 0, 0:RANK],
                    rhs=tT_all[:, 0, :],
                    start=True,
                    stop=True,
                )

            # Startup: interleave the four chunk-0 m-groups slab-wise so PE
            # consumption matches DMA delivery; t-phase(0) rides along.
            yps = [new_ypsum() for _ in range(MPC)]
            for ms in range(MPC):
                mg_8(0, ms, yps[ms])
            t_part(0, 0)
            for q in range(1, 4):
                for ms in range(MPC):
                    mg_16(0, ms, yps[ms], (q - 1) * KQ, q * KQ)
                t_part(0, q)
            for ms in range(MPC):
                mg_tail(0, ms, yps[ms])

            yp04 = new_ypsum()
            mg_8(0, 4, yp04)
            t_part(1, 0)
            mg_16(0, 4, yp04, 0, KR // 2)
            t_part(1, 1)
            t_part(1, 2)
            mg_16(0, 4, yp04, KR // 2, KR)
            t_part(1, 3)
            mg_tail(0, 4, yp04)

            for ms in range(5, MS):
                mg_full(0, ms)
            for ws in range(1, NS):
                for ms in range(MS):
                    mg_full(ws, ms)

    nc.compile()
    return nc


def _get_graph():
    global _graph_cache
    if _graph_cache is None:
        _graph_cache = _build_graph()
    return _graph_cache


def _prep_inputs(inputs):
    """Host-side: fold scales, cast to fp8/bf16, pre-tile to SBUF layout."""
    x = np.asarray(inputs["x"], dtype=np.float32)
    weight = np.asarray(inputs["weight"], dtype=np.float32)
    P = np.asarray(inputs["P"], dtype=np.float32)
    Lambda = np.asarray(inputs["Lambda"], dtype=np.float32)
    Q = np.asarray(inputs["Q"], dtype=np.float32)
    rank_mask = np.asarray(inputs["rank_mask"])

    KFE = KF * P_DIM  # 1024 leading k-elements in fp8

    scale = (SCALING * Lambda * rank_mask.astype(np.float32)).astype(np.float32)
    ptil = (P * scale[None, :]).T * (WSCALE / QSCALE)  # [RANK, OUT_F]
    pt = np.zeros((P_DIM, OUT_F), dtype=BF16)
    pt[:RANK] = ptil.astype(BF16)

    q64 = (Q * QSCALE).T  # [IN_F, RANK]
    qt8 = np.ascontiguousarray(
        q64[:KFE].astype(F8).reshape(KFP, 2, P_DIM, RANK).transpose(2, 0, 1, 3)
    )
    qt16 = np.ascontiguousarray(
        q64[KFE:].astype(BF16).reshape(KR, P_DIM, RANK).transpose(1, 0, 2)
    )

    ws = (weight * WSCALE).T  # [IN_F, OUT_F]
    w8 = np.ascontiguousarray(
        ws[:KFE]
        .astype(F8)
        .reshape(KFP, 2, P_DIM, NS, N_STRIPE)
        .transpose(3, 2, 0, 1, 4)
    )
    w16 = np.ascontiguousarray(
        ws[KFE:]
        .astype(BF16)
        .reshape(KR, P_DIM, NS, N_STRIPE)
        .transpose(2, 1, 0, 3)
    )

    in_maps = []
    for c in range(N_CORES):
        xc = x[c * M_PER : (c + 1) * M_PER]  # [1024, 4096]
        xct = xc.T  # [IN_F, 1024]
        x8c = np.ascontiguousarray(
            xct[:KFE]
            .astype(F8)
            .reshape(KFP, 2, P_DIM, NXC, XH)
            .transpose(3, 2, 0, 1, 4)
        )
        x16c = np.ascontiguousarray(
            xct[KFE:]
            .astype(BF16)
            .reshape(KR, P_DIM, NXC, XH)
            .transpose(2, 1, 0, 3)
        )
        in_maps.append(
            {
                "x8": x8c,
                "x16": x16c,
                "w8": w8,
                "w16": w16,
                "qt8": qt8,
                "qt16": qt16,
                "pt": pt,
            }
        )
    return in_maps


def run_full(inputs, trace=False, trace_kwargs=None):
    """Run the SPMD kernel on 8 cores. Returns (y_full, BassKernelResults)."""
    in_maps = _prep_inputs(inputs)

    nc = _get_graph()
    last_err = None
    for attempt in range(3):
        try:
            res = run_bass_kernel_spmd(
                nc,
                in_maps,
                core_ids=list(range(N_CORES)),
                trace=trace,
                **(trace_kwargs or {}),
            )
            break
        except Exception as e:
            last_err = e
            time.sleep(10)
    else:
        raise last_err
    y = np.concatenate([res.results[c]["out"] for c in range(N_CORES)], axis=0)
    return y.astype(np.float32, copy=False), res


def _device_available():
    try:
        import jax

        return any("NC" in str(d) or "axon" in str(d).lower() for d in jax.devices())
    except Exception:
        return False


def _run_in_subprocess(inputs):
    import pickle
    import subprocess
    import tempfile

    with tempfile.TemporaryDirectory() as td:
        in_path = os.path.join(td, "in.pkl")
        out_path = os.path.join(td, "out.npy")
        with open(in_path, "wb") as f:
            pickle.dump({k: np.asarray(v) for k, v in inputs.items()}, f)
        env = dict(os.environ)
        env.pop("JAX_PLATFORMS", None)
        env["KERNEL_NO_SUBPROC"] = "1"
        code = (
            "import sys, pickle, numpy as np; "
            f"sys.path.insert(0, {os.path.dirname(os.path.abspath(__file__))!r}); "
            "import kernel; "
            f"inputs = pickle.load(open({in_path!r}, 'rb')); "
            "y, _ = kernel.run_full(inputs, trace=False); "
            f"np.save({out_path!r}, y)"
        )
        subprocess.run([sys.executable, "-c", code], env=env, check=True)
        return np.load(out_path)


def kernel(**inputs) -> np.ndarray:
    if os.environ.get("KERNEL_NO_SUBPROC") != "1":
        if not _device_available():
            return _run_in_subprocess(inputs)
        try:
            y, _ = run_full(inputs, trace=False)
            return y
        except Exception:
            return _run_in_subprocess(inputs)
    y, _ = run_full(inputs, trace=False)
    return y


# revision 14
# speedup vs baseline: 1.1937x; 1.0368x over previous
"""AdaLoRA linear layer on 8 TRN2 NeuronCores — mixed fp8/bf16 PE path.

Computes y = x @ (W + s * (P*Lambda*mask) @ Q)^T for
x[8192,4096], W[4096,4096], P[4096,64], Q[64,4096], s=2.0.

Data-parallel over tokens (1024/core). The contraction dim is split:
the first KF=8 k-blocks (1024 of 4096) run as fp8e4 DoubleRow matmuls
(2 k-blocks per instruction, 2x PE throughput — measured 216ns per
K=256 x 512 instr, same as one bf16 K=128 instr), the remaining 24
k-blocks run in bf16. Measured end-to-end rel err 1.59e-2 on the
reference inputs (gate: 2e-2); fp8 quantization error scales with
sqrt(KF/KB) so KF=8 keeps a >20% margin.

Scale folding so one PSUM accumulation group stays consistent:
  W is pre-scaled x32 on both the fp8 and bf16 sides (fp8 needs it to
  stay in e4m3 normal range; bf16 absorbs it exactly), Q x64, and
  Ptilde = P*(s*Lambda*mask) enters as Ptilde*32/64; the final
  psum->SBUF copy multiplies by 1/32 on the Activation engine.
"""

import os
import sys
import time
import types

for _p in ("/opt/trn_rl_repo", "/opt/pypackages"):
    if os.path.isdir(_p) and _p not in sys.path:
        sys.path.append(_p)

try:
    import antenv.axon_hooks  # noqa: F401
except Exception:
    _mod = types.ModuleType("antenv.axon_hooks")
    _mod._hook = None

    def _set_hook(h, _m=_mod):
        _m._hook = h

    def _get_hook(_m=_mod):
        return _m._hook

    _mod.set_axon_ntff_profile_hook = _set_hook
    _mod.get_axon_ntff_profile_hook = _get_hook
    try:
        from trn_agent_boot.trn_boot import _ntff_profile_via_ctypes

        _mod._hook = _ntff_profile_via_ctypes("/opt/axon/libaxon_pjrt.so")
    except Exception:
        pass
    sys.modules["antenv.axon_hooks"] = _mod

import ml_dtypes
import numpy as np

import concourse.mybir as mybir
import concourse.tile as tile
from concourse import bacc
from concourse.bass_utils import run_bass_kernel_spmd
from concourse.tile_rust import add_dep_helper

N_CORES = 8
IN_F = 4096
OUT_F = 4096
RANK = 64
BT = 8192
M_PER = BT // N_CORES
SCALING = 2.0

P_DIM = 128
KB = IN_F // P_DIM  # 32 k-blocks
KF = 10  # k-blocks in fp8 DoubleRow (must be even)
KFP = KF // 2  # DR instructions per group
KR = KB - KF  # bf16 k-blocks
# bf16 kb-range split into 3 DMA slabs (also the startup interleave
# granularity); sizes need not be equal.
KSL = [(0, KR // 3), (KR // 3, 2 * KR // 3), (2 * KR // 3, KR)]
MS = M_PER // P_DIM
N_STRIPE = 512
NS = OUT_F // N_STRIPE

XH = 512
NXC = M_PER // XH
MPC = XH // P_DIM

WSCALE = 32.0
QSCALE = 64.0

BF16 = ml_dtypes.bfloat16
F8 = ml_dtypes.float8_e4m3

_graph_cache = None


def _build_graph():
    f32 = mybir.dt.float32
    bf16 = mybir.dt.bfloat16
    f8 = mybir.dt.float8e4
    DR = mybir.MatmulPerfMode.DoubleRow

    nc = bacc.Bacc(None, target_bir_lowering=False, debug=False)

    x8d = nc.declare_dram_parameter("x8", [NXC, P_DIM, KFP, 2, XH], f8, isOutput=False)
    x16d = nc.declare_dram_parameter("x16", [NXC, P_DIM, KR, XH], bf16, isOutput=False)
    w8d = nc.declare_dram_parameter(
        "w8", [NS, P_DIM, KFP, 2, N_STRIPE], f8, isOutput=False
    )
    w16d = nc.declare_dram_parameter(
        "w16", [NS, P_DIM, KR, N_STRIPE], bf16, isOutput=False
    )
    qt8d = nc.declare_dram_parameter("qt8", [P_DIM, KFP, 2, RANK], f8, isOutput=False)
    qt16d = nc.declare_dram_parameter("qt16", [P_DIM, KR, RANK], bf16, isOutput=False)
    ptd = nc.declare_dram_parameter("pt", [P_DIM, OUT_F], bf16, isOutput=False)
    out = nc.declare_dram_parameter("out", [M_PER, OUT_F], f32, isOutput=True)

    with tile.TileContext(nc) as tc:
        with (
            tc.tile_pool(name="const", bufs=1) as constp,
            tc.tile_pool(name="xpool", bufs=1) as xpool,
            tc.tile_pool(name="wpool", bufs=2) as wpool,
            tc.tile_pool(name="ypool", bufs=3) as ypool,
            tc.tile_pool(name="psum_y", bufs=6, space="PSUM") as psum_y_pool,
            tc.tile_pool(name="psum_t", bufs=2, space="PSUM") as psum_t_pool,
        ):
            # ---- tiles ----
            qt8_sb = constp.tile([P_DIM, KFP, 2, RANK], f8)
            qt16_sb = constp.tile([P_DIM, KR, RANK], bf16)
            pt_sb = constp.tile([P_DIM, OUT_F], bf16)
            tT_all = constp.tile([P_DIM, NXC, XH], bf16)
            nc.vector.memset(tT_all[:], 0.0)

            x8s, x16s = [], []
            for h in range(NXC):
                x8_h = xpool.tile(
                    [P_DIM, KFP, 2, XH], f8, name=f"x8_h{h}", tag=f"x8_h{h}"
                )
                x16_h = xpool.tile(
                    [P_DIM, KR, XH], bf16, name=f"x16_h{h}", tag=f"x16_h{h}"
                )
                x8s.append(x8_h)
                x16s.append(x16_h)
            w8s, w16s = [], []
            for ns in range(NS):
                w8_sb = wpool.tile(
                    [P_DIM, KFP, 2, N_STRIPE], f8, tag="w8_sb", name=f"w8_sb{ns}"
                )
                w16_sb = wpool.tile(
                    [P_DIM, KR, N_STRIPE], bf16, tag="w16_sb", name=f"w16_sb{ns}"
                )
                w8s.append(w8_sb)
                w16s.append(w16_sb)

            # ---- input DMAs on gpsimd, paced pairwise with the PE's
            # kb-slab consumption during startup ----

            def x8slab(h):
                return nc.gpsimd.dma_start(out=x8s[h][:], in_=x8d[h])

            def x16slab(h, s_):
                sl = slice(*KSL[s_])
                return nc.gpsimd.dma_start(
                    out=x16s[h][:, sl, :], in_=x16d[h, :, sl, :]
                )

            def w8slab(ns):
                return nc.gpsimd.dma_start(out=w8s[ns][:], in_=w8d[ns])

            def w16slab(ns, s_):
                sl = slice(*KSL[s_])
                return nc.gpsimd.dma_start(
                    out=w16s[ns][:, sl, :], in_=w16d[ns, :, sl, :]
                )

            dma_qt8 = nc.gpsimd.dma_start(out=qt8_sb[:], in_=qt8d[:])
            dma_qt16 = nc.gpsimd.dma_start(out=qt16_sb[:], in_=qt16d[:])
            x0 = [x8slab(0), None, None, None]
            w0 = [w8slab(0), None, None, None]
            for s_ in range(3):
                x0[s_ + 1] = x16slab(0, s_)
                w0[s_ + 1] = w16slab(0, s_)
                add_dep_helper(x0[s_ + 1].ins, x0[s_].ins, reason="pace x0")
            x1 = [x8slab(1), x16slab(1, 0)]
            add_dep_helper(x1[0].ins, x0[2].ins, reason="pace x1")
            w1 = [w8slab(1), w16slab(1, 0)]
            add_dep_helper(w1[0].ins, w0[3].ins, reason="pace w1")
            x1 += [x16slab(1, 1), x16slab(1, 2)]
            add_dep_helper(x1[2].ins, x0[3].ins, reason="pace x1c")
            dma_pt = nc.gpsimd.dma_start(out=pt_sb[:], in_=ptd[:])
            add_dep_helper(dma_pt.ins, x1[0].ins, reason="pace pt")
            w1 += [w16slab(1, 1), w16slab(1, 2)]
            add_dep_helper(w1[2].ins, w1[0].ins, reason="pace w1c")
            wtail = list(w1)
            for ns in range(2, NS):
                for d in (
                    w8slab(ns),
                    w16slab(ns, 0),
                    w16slab(ns, 1),
                    w16slab(ns, 2),
                ):
                    add_dep_helper(
                        d.ins, wtail[-3].ins, reason="dma window order"
                    )
                    wtail.append(d)

            # ---- compute ----
            t_psums = [None, None]

            def t_part(h, part):
                # part 0: fp8 DR k-blocks; 1/2/3: bf16 kb slabs; 3 closes.
                if part == 0:
                    t_psums[h] = psum_t_pool.tile(
                        [RANK, XH], f32, tag="psum_tT", name=f"psum_tT{h}"
                    )
                    for j in range(KFP):
                        nc.tensor.matmul(
                            t_psums[h][:],
                            lhsT=qt8_sb[:, j, :, :],
                            rhs=x8s[h][:, j, :, :],
                            start=(j == 0),
                            stop=False,
                            perf_mode=DR,
                        )
                else:
                    for kb in range(*KSL[part - 1]):
                        nc.tensor.matmul(
                            t_psums[h][:],
                            lhsT=qt16_sb[:, kb, :],
                            rhs=x16s[h][:, kb, :],
                            start=False,
                            stop=(kb == KR - 1),
                        )
                    if part == 3:
                        nc.scalar.copy(out=tT_all[0:RANK, h, :], in_=t_psums[h][:])

            def mg_8(ws, ms, ypsum):
                h, mo = divmod(ms, MPC)
                msl = slice(mo * P_DIM, (mo + 1) * P_DIM)
                for j in range(KFP):
                    nc.tensor.matmul(
                        ypsum[:],
                        lhsT=x8s[h][:, j, :, msl],
                        rhs=w8s[ws][:, j, :, :],
                        start=(j == 0),
                        stop=False,
                        perf_mode=DR,
                    )

            def mg_16(ws, ms, ypsum, k0, k1):
                h, mo = divmod(ms, MPC)
                msl = slice(mo * P_DIM, (mo + 1) * P_DIM)
                for kb in range(k0, k1):
                    nc.tensor.matmul(
                        ypsum[:],
                        lhsT=x16s[h][:, kb, msl],
                        rhs=w16s[ws][:, kb, :],
                        start=False,
                        stop=False,
                    )

            def mg_tail(ws, ms, ypsum):
                h, mo = divmod(ms, MPC)
                msl = slice(mo * P_DIM, (mo + 1) * P_DIM)
                off = ws * N_STRIPE
                nc.tensor.matmul(
                    ypsum[:],
                    lhsT=tT_all[:, h, msl],
                    rhs=pt_sb[:, off : off + N_STRIPE],
                    start=False,
                    stop=True,
                )
                y_sb = ypool.tile([P_DIM, N_STRIPE], f32, tag="y_sb", name="y_sb")
                nc.scalar.mul(y_sb[:], ypsum[:], 1.0 / WSCALE)
                nc.sync.dma_start(
                    out=out[ms * P_DIM : (ms + 1) * P_DIM, off : off + N_STRIPE],
                    in_=y_sb[:],
                )

            def new_ypsum():
                return psum_y_pool.tile(
                    [P_DIM, N_STRIPE], f32, tag="ypsum", name="ypsum"
                )

            def mg_full(ws, ms):
                yp = new_ypsum()
                mg_8(ws, ms, yp)
                mg_16(ws, ms, yp, 0, KR)
                mg_tail(ws, ms, yp)

            # PE pre-warm on the zeroed tT region while first DMAs fly.
            warm_ps = psum_t_pool.tile(
                [RANK, XH], f32, tag="psum_tT", name="warm_ps"
            )
            for _ in range(12):
                nc.tensor.matmul(
                    warm_ps[:],
                    lhsT=tT_all[:, 0, 0:RANK],
                    rhs=tT_all[:, 0, :],
                    start=True,
                    stop=True,
                )

            # Startup: interleave the four chunk-0 m-groups slab-wise so PE
            # consumption matches DMA delivery; t-phase(0) rides along.
            yps = [new_ypsum() for _ in range(MPC)]
            for ms in range(MPC):
                mg_8(0, ms, yps[ms])
            t_part(0, 0)
            for q in range(1, 4):
                for ms in range(MPC):
                    mg_16(0, ms, yps[ms], *KSL[q - 1])
                t_part(0, q)
            for ms in range(MPC):
                mg_tail(0, ms, yps[ms])

            yp04 = new_ypsum()
            mg_8(0, 4, yp04)
            t_part(1, 0)
            mg_16(0, 4, yp04, 0, KR // 2)
            t_part(1, 1)
            t_part(1, 2)
            mg_16(0, 4, yp04, KR // 2, KR)
            t_part(1, 3)
            mg_tail(0, 4, yp04)

            for ms in range(5, MS):
                mg_full(0, ms)
            for ws in range(1, NS):
                for ms in range(MS):
                    mg_full(ws, ms)

    nc.compile()
    return nc


def _get_graph():
    global _graph_cache
    if _graph_cache is None:
        _graph_cache = _build_graph()
    return _graph_cache


def _prep_inputs(inputs):
    """Host-side: fold scales, cast to fp8/bf16, pre-tile to SBUF layout."""
    x = np.asarray(inputs["x"], dtype=np.float32)
    weight = np.asarray(inputs["weight"], dtype=np.float32)
    P = np.asarray(inputs["P"], dtype=np.float32)
    Lambda = np.asarray(inputs["Lambda"], dtype=np.float32)
    Q = np.asarray(inputs["Q"], dtype=np.float32)
    rank_mask = np.asarray(inputs["rank_mask"])

    KFE = KF * P_DIM  # 1024 leading k-elements in fp8

    scale = (SCALING * Lambda * rank_mask.astype(np.float32)).astype(np.float32)
    ptil = (P * scale[None, :]).T * (WSCALE / QSCALE)  # [RANK, OUT_F]
    pt = np.zeros((P_DIM, OUT_F), dtype=BF16)
    pt[:RANK] = ptil.astype(BF16)

    q64 = (Q * QSCALE).T  # [IN_F, RANK]
    qt8 = np.ascontiguousarray(
        q64[:KFE].astype(F8).reshape(KFP, 2, P_DIM, RANK).transpose(2, 0, 1, 3)
    )
    qt16 = np.ascontiguousarray(
        q64[KFE:].astype(BF16).reshape(KR, P_DIM, RANK).transpose(1, 0, 2)
    )

    ws = (weight * WSCALE).T  # [IN_F, OUT_F]
    w8 = np.ascontiguousarray(
        ws[:KFE]
        .astype(F8)
        .reshape(KFP, 2, P_DIM, NS, N_STRIPE)
        .transpose(3, 2, 0, 1, 4)
    )
    w16 = np.ascontiguousarray(
        ws[KFE:]
        .astype(BF16)
        .reshape(KR, P_DIM, NS, N_STRIPE)
        .transpose(2, 1, 0, 3)
    )

    in_maps = []
    for c in range(N_CORES):
        xc = x[c * M_PER : (c + 1) * M_PER]  # [1024, 4096]
        xct = xc.T  # [IN_F, 1024]
        x8c = np.ascontiguousarray(
            xct[:KFE]
            .astype(F8)
            .reshape(KFP, 2, P_DIM, NXC, XH)
            .transpose(3, 2, 0, 1, 4)
        )
        x16c = np.ascontiguousarray(
            xct[KFE:]
            .astype(BF16)
            .reshape(KR, P_DIM, NXC, XH)
            .transpose(2, 1, 0, 3)
        )
        in_maps.append(
            {
                "x8": x8c,
                "x16": x16c,
                "w8": w8,
                "w16": w16,
                "qt8": qt8,
                "qt16": qt16,
                "pt": pt,
            }
        )
    return in_maps


def run_full(inputs, trace=False, trace_kwargs=None):
    """Run the SPMD kernel on 8 cores. Returns (y_full, BassKernelResults)."""
    in_maps = _prep_inputs(inputs)

    nc = _get_graph()
    last_err = None
    for attempt in range(3):
        try:
            res = run_bass_kernel_spmd(
                nc,
                in_maps,
                core_ids=list(range(N_CORES)),
                trace=trace,
                **(trace_kwargs or {}),
            )
            break
        except Exception as e:
            last_err = e
            time.sleep(10)
    else:
        raise last_err
    y = np.concatenate([res.results[c]["out"] for c in range(N_CORES)], axis=0)
    return y.astype(np.float32, copy=False), res


def _device_available():
    try:
        import jax

        return any("NC" in str(d) or "axon" in str(d).lower() for d in jax.devices())
    except Exception:
        return False


def _run_in_subprocess(inputs):
    import pickle
    import subprocess
    import tempfile

    with tempfile.TemporaryDirectory() as td:
        in_path = os.path.join(td, "in.pkl")
        out_path = os.path.join(td, "out.npy")
        with open(in_path, "wb") as f:
            pickle.dump({k: np.asarray(v) for k, v in inputs.items()}, f)
        env = dict(os.environ)
        env.pop("JAX_PLATFORMS", None)
        env["KERNEL_NO_SUBPROC"] = "1"
        code = (
            "import sys, pickle, numpy as np; "
            f"sys.path.insert(0, {os.path.dirname(os.path.abspath(__file__))!r}); "
            "import kernel; "
            f"inputs = pickle.load(open({in_path!r}, 'rb')); "
            "y, _ = kernel.run_full(inputs, trace=False); "
            f"np.save({out_path!r}, y)"
        )
        subprocess.run([sys.executable, "-c", code], env=env, check=True)
        return np.load(out_path)


def kernel(**inputs) -> np.ndarray:
    if os.environ.get("KERNEL_NO_SUBPROC") != "1":
        if not _device_available():
            return _run_in_subprocess(inputs)
        try:
            y, _ = run_full(inputs, trace=False)
            return y
        except Exception:
            return _run_in_subprocess(inputs)
    y, _ = run_full(inputs, trace=False)
    return y
